# revision 7
# baseline (speedup 1.0000x reference)
"""Additive-attention (tanh energy + softmax + context) kernel for 8 TRN2 NeuronCores.

Data-parallel over the batch dim B=128 -> 16 samples per core.

Per-core device pipeline (per sample b):
  pre.T[o, t]  = W_h @ X_pos[b].T + W_d @ X_dyn[b].T      (PE, bf16, f32 psum)
  z.T[o, t]    = tanh(pre.T)                              (ACT, -> bf16 sbuf)
  energy[t, 1] = z.T(chunk).T @ W_a                       (PE, column layout [128, 16])
  es           = (energy + b_a) * scale                   (DVE)
  exp, rowsum  = Exp activation with accum_out            (ACT)
  total        = ones-matmul partition reduce + broadcast (PE)
  alpha        = exp * (1/total)                          (DVE)
  alphaT       = PE transpose -> DMA out  [T] f32
  context[1,h] = sum_j alpha[:,j].T @ X_pos_nat[jj]       (PE, accumulate in psum)

Host side: shard B across 8 cores, cast X to bf16, pre-transpose layouts so every
device DMA is fully contiguous, run SPMD, gather outputs.
"""

import numpy as np
import ml_dtypes

B, T, H = 128, 2048, 128
N_CORES = 8
B_LOC = B // N_CORES      # 16 samples per core
TCH = 512                 # t-chunk for the pre matmuls (one psum bank)
NCH = T // TCH            # 4 chunks per sample
NJJ = T // 128            # 16 energy columns per sample

_BF16 = ml_dtypes.bfloat16

_cache = {}


def _build_bass(n_samples=B_LOC):
    import concourse.bass as bass
    import concourse.tile as tile
    from concourse import bacc, mybir
    from contextlib import ExitStack

    f32 = mybir.dt.float32
    bf16 = mybir.dt.bfloat16
    AF = mybir.ActivationFunctionType
    OP = mybir.AluOpType

    nc = bacc.Bacc(
        "TRN2", target_bir_lowering=False, debug=False, num_devices=N_CORES
    )

    xposT = nc.dram_tensor("xposT", [B_LOC, H, T], bf16, kind="ExternalInput").ap()
    xdynT = nc.dram_tensor("xdynT", [B_LOC, H, T], bf16, kind="ExternalInput").ap()
    xposN = nc.dram_tensor("xposN", [B_LOC, 128, T], bf16, kind="ExternalInput").ap()
    scale_c = nc.dram_tensor("scale_c", [B_LOC, 128, NJJ], f32, kind="ExternalInput").ap()
    whT = nc.dram_tensor("whT", [H, H], bf16, kind="ExternalInput").ap()
    wdT = nc.dram_tensor("wdT", [H, H], bf16, kind="ExternalInput").ap()
    wa = nc.dram_tensor("wa", [H, 1], bf16, kind="ExternalInput").ap()
    ident = nc.dram_tensor("ident", [128, 128], f32, kind="ExternalInput").ap()
    ones_col = nc.dram_tensor("ones_col", [128, 1], f32, kind="ExternalInput").ap()
    ones_row = nc.dram_tensor("ones_row", [1, 128], f32, kind="ExternalInput").ap()
    ba_col = nc.dram_tensor("ba_col", [128, 1], f32, kind="ExternalInput").ap()

    alpha_out = nc.dram_tensor("alpha_out", [B_LOC, T], f32, kind="ExternalOutput").ap()
    ctx_out = nc.dram_tensor("ctx_out", [B_LOC, H], f32, kind="ExternalOutput").ap()

    with tile.TileContext(nc) as tc, ExitStack() as ctx:
        consts = ctx.enter_context(tc.tile_pool(name="consts", bufs=1))
        xt_pool = ctx.enter_context(tc.tile_pool(name="xt", bufs=3))
        xd_pool = ctx.enter_context(tc.tile_pool(name="xd", bufs=3))
        xn_pool = ctx.enter_context(tc.tile_pool(name="xn", bufs=3))
        z_pool = ctx.enter_context(tc.tile_pool(name="z", bufs=4))
        sm_pool = ctx.enter_context(tc.tile_pool(name="sm", bufs=4))
        ps_pre = ctx.enter_context(tc.tile_pool(name="pspre", bufs=2, space="PSUM"))
        ps_e = ctx.enter_context(tc.tile_pool(name="pse", bufs=2, space="PSUM"))
        ps_sm = ctx.enter_context(tc.tile_pool(name="pssm", bufs=3, space="PSUM"))

        whT_sb = consts.tile([H, H], bf16, tag="whT")
        nc.sync.dma_start(whT_sb[:], whT[:])
        wdT_sb = consts.tile([H, H], bf16, tag="wdT")
        nc.sync.dma_start(wdT_sb[:], wdT[:])
        wa_sb = consts.tile([H, 1], bf16, tag="wa")
        nc.sync.dma_start(wa_sb[:], wa[:])
        ident_sb = consts.tile([128, 128], f32, tag="ident")
        nc.sync.dma_start(ident_sb[:], ident[:])
        onesc_sb = consts.tile([128, 1], f32, tag="onesc")
        nc.sync.dma_start(onesc_sb[:], ones_col[:])
        onesr_sb = consts.tile([1, 128], f32, tag="onesr")
        nc.sync.dma_start(onesr_sb[:], ones_row[:])
        ba_sb = consts.tile([128, 1], f32, tag="ba")
        nc.sync.dma_start(ba_sb[:], ba_col[:])

        for b in range(n_samples):
            xt = xt_pool.tile([H, T], bf16, tag="xt")
            nc.sync.dma_start(xt[:], xposT[b])
            xd = xd_pool.tile([H, T], bf16, tag="xd")
            nc.sync.dma_start(xd[:], xdynT[b])
            xn = xn_pool.tile([128, T], bf16, tag="xn")
            nc.sync.dma_start(xn[:], xposN[b])
            sc = sm_pool.tile([128, NJJ], f32, tag="sc")
            nc.sync.dma_start(sc[:], scale_c[b])

            pe_ps = ps_e.tile([128, NJJ], f32, tag="pe")
            for c in range(NCH):
                pp = ps_pre.tile([128, TCH], f32, tag="pp")
                nc.tensor.matmul(
                    pp[:], lhsT=whT_sb[:], rhs=xt[:, c * TCH:(c + 1) * TCH],
                    start=True, stop=False,
                )
                nc.tensor.matmul(
                    pp[:], lhsT=wdT_sb[:], rhs=xd[:, c * TCH:(c + 1) * TCH],
                    start=False, stop=True,
                )
                zz = z_pool.tile([128, TCH], bf16, tag="zz")
                nc.scalar.activation(zz[:], pp[:], AF.Tanh)
                for q in range(TCH // 128):
                    jj = c * (TCH // 128) + q
                    nc.tensor.matmul(
                        pe_ps[:, jj:jj + 1],
                        lhsT=zz[:, q * 128:(q + 1) * 128],
                        rhs=wa_sb[:],
                        start=True, stop=True,
                    )

            # es = (energy + b_a) * scale
            es = sm_pool.tile([128, NJJ], f32, tag="es")
            nc.vector.scalar_tensor_tensor(
                out=es[:], in0=pe_ps[:], scalar=ba_sb[:], in1=sc[:],
                op0=OP.add, op1=OP.mult,
            )
            # exp + per-partition row sums
            ex = sm_pool.tile([128, NJJ], f32, tag="ex")
            exs = sm_pool.tile([128, 1], f32, tag="exs")
            nc.scalar.activation(ex[:], es[:], AF.Exp, accum_out=exs[:])
            # total over partitions, then broadcast back to 128 partitions
            ptot = ps_sm.tile([1, 1], f32, tag="pssm")
            nc.tensor.matmul(ptot[:], lhsT=exs[:], rhs=onesc_sb[:], start=True, stop=True)
            tot_sb = sm_pool.tile([1, 1], f32, tag="tot")
            nc.scalar.copy(tot_sb[:], ptot[:])
            pbc = ps_sm.tile([128, 1], f32, tag="pssm")
            nc.tensor.matmul(pbc[:], lhsT=onesr_sb[:], rhs=tot_sb[:], start=True, stop=True)
            rt = sm_pool.tile([128, 1], f32, tag="rt")
            nc.vector.reciprocal(rt[:], pbc[:])
            # alpha (f32 for output, bf16 for context weights)
            al = sm_pool.tile([128, NJJ], f32, tag="al")
            nc.vector.tensor_scalar(
                out=al[:], in0=ex[:], scalar1=rt[:], scalar2=None, op0=OP.mult,
            )
            ab = sm_pool.tile([128, NJJ], bf16, tag="ab")
            nc.vector.tensor_copy(ab[:], al[:])
            # alpha.T -> [16, 128] -> DMA out
            pat = ps_sm.tile([NJJ, 128], f32, tag="pssm")
            nc.tensor.transpose(pat[:], al[:], ident_sb[:])
            at = sm_pool.tile([NJJ, 128], f32, tag="at")
            nc.scalar.copy(at[:], pat[:])
            nc.sync.dma_start(
                alpha_out[b].rearrange("(j u) -> j u", j=NJJ), at[:]
            )
            # context: accumulate sum_j alpha_col_j.T @ Xnat_j -> [1, H]
            pctx = ps_sm.tile([1, H], f32, tag="pssm")
            for jj in range(NJJ):
                nc.tensor.matmul(
                    pctx[:],
                    lhsT=ab[:, jj:jj + 1],
                    rhs=xn[:, jj * 128:(jj + 1) * 128],
                    start=(jj == 0), stop=(jj == NJJ - 1),
                )
            ctx_row = sm_pool.tile([1, H], f32, tag="ctxrow")
            nc.scalar.copy(ctx_row[:], pctx[:])
            nc.sync.dma_start(ctx_out[b].rearrange("(o h) -> o h", o=1), ctx_row[:])

    nc.compile()
    return nc


def _get_nc():
    if "nc" not in _cache:
        _cache["nc"] = _build_bass()
    return _cache["nc"]


def _prep_core_inputs(Hp_bf, Hd_bf, scale, b_a):
    """Build the per-core input maps (host-side layout transforms)."""
    ident = np.eye(128, dtype=np.float32)
    ones_col = np.ones((128, 1), np.float32)
    ones_row = np.ones((1, 128), np.float32)
    ba_col = np.full((128, 1), np.float32(b_a), np.float32)
    in_maps = []
    for core in range(N_CORES):
        sl = slice(core * B_LOC, (core + 1) * B_LOC)
        hp = Hp_bf[sl]                       # [16, T, H] bf16
        hd = Hd_bf[sl]
        in_maps.append({
            "xposT": np.ascontiguousarray(hp.transpose(0, 2, 1)),
            "xdynT": np.ascontiguousarray(hd.transpose(0, 2, 1)),
            # [b, (j p), h] -> [b, p, (j h)]
            "xposN": np.ascontiguousarray(
                hp.reshape(B_LOC, NJJ, 128, H).transpose(0, 2, 1, 3)
            ).reshape(B_LOC, 128, T),
            # [b, (j p)] -> [b, p, j]
            "scale_c": np.ascontiguousarray(
                scale[sl].reshape(B_LOC, NJJ, 128).transpose(0, 2, 1)
            ),
            "whT": _cache["whT"],
            "wdT": _cache["wdT"],
            "wa": _cache["wa"],
            "ident": ident,
            "ones_col": ones_col,
            "ones_row": ones_row,
            "ba_col": ba_col,
        })
    return in_maps


def kernel(H_pos, H_dyn, acc_w, W_h, W_d, W_a, b_a, beta):
    from concourse.bass_utils import run_bass_kernel_spmd

    H_pos = np.asarray(H_pos, dtype=np.float32)
    H_dyn = np.asarray(H_dyn, dtype=np.float32)
    acc_w = np.asarray(acc_w, dtype=np.float32)
    W_h = np.asarray(W_h, dtype=np.float32)
    W_d = np.asarray(W_d, dtype=np.float32)
    W_a = np.asarray(W_a, dtype=np.float32)
    b_a_f = float(np.asarray(b_a))
    beta_f = float(np.asarray(beta))

    # host scalar/row prep (tiny): softplus(beta), acc normalization, scale
    beta_pos = float(np.log1p(np.exp(beta_f)))
    acc_norm = acc_w / np.clip(acc_w.max(axis=1, keepdims=True), 1e-6, None)
    scale = (1.0 + beta_pos * acc_norm).astype(np.float32)          # [B, T]

    Hp_bf = H_pos.astype(_BF16)
    Hd_bf = H_dyn.astype(_BF16)
    _cache["whT"] = np.ascontiguousarray(W_h.T).astype(_BF16)
    _cache["wdT"] = np.ascontiguousarray(W_d.T).astype(_BF16)
    _cache["wa"] = W_a.reshape(H, 1).astype(_BF16)

    nc = _get_nc()
    in_maps = _prep_core_inputs(Hp_bf, Hd_bf, scale, b_a_f)
    res = run_bass_kernel_spmd(nc, in_maps, list(range(N_CORES)))
    _cache["last_res"] = res

    alpha = np.concatenate([r["alpha_out"] for r in res.results], axis=0)
    context = np.concatenate([r["ctx_out"] for r in res.results], axis=0)
    return (
        context.astype(np.float32, copy=False),
        alpha.astype(np.float32, copy=False),
    )


# revision 8
# speedup vs baseline: 1.4076x; 1.4076x over previous
"""Additive-attention (tanh energy + softmax + context) kernel for 8 TRN2 NeuronCores.

Data-parallel over the batch dim B=128 -> 16 samples per core.

Three-phase per-core pipeline (keeps the PE dense / warm):
  Phase 1 (per sample, pipelined):
    pre.T[o, t]  = W_h @ X_pos[b].T + W_d @ X_dyn[b].T    (PE, bf16, f32 psum)
    z.T[o, t]    = tanh(pre.T)                            (ACT -> bf16 sbuf)
    energy cols  = z.T(chunk).T @ W_a -> psum [128, 16]   (PE)
    es_all[:,b,:] <- psum_e                               (DVE copy)
  Phase 2 (batched over all 16 samples):
    es = (e + b_a) * scale; exp; per-sample sums; totals via ones-matmul;
    transpose+outer-product broadcast; alpha = exp * 1/total
  Phase 3 (per sample):
    alpha.T -> DMA out; context = sum_j alpha_col_j.T @ X_nat_j -> DMA out

Host side: shard B across 8 cores, cast X to bf16, pre-transpose layouts so every
device DMA is fully contiguous, run SPMD, gather outputs.
"""

import numpy as np
import ml_dtypes

B, T, H = 128, 2048, 128
N_CORES = 8
B_LOC = B // N_CORES      # 16 samples per core
TCH = 512                 # t-chunk for the pre matmuls (one psum bank)
NCH = T // TCH            # 4 chunks per sample
NJJ = T // 128            # 16 energy columns per sample
XBAT = 4                  # samples per xposT/xdynT DMA batch

_BF16 = ml_dtypes.bfloat16

_cache = {}


def _build_bass(n_samples=B_LOC):
    import concourse.bass as bass
    import concourse.tile as tile
    from concourse import bacc, mybir
    from contextlib import ExitStack

    f32 = mybir.dt.float32
    bf16 = mybir.dt.bfloat16
    AF = mybir.ActivationFunctionType
    OP = mybir.AluOpType

    nc = bacc.Bacc(
        "TRN2", target_bir_lowering=False, debug=False, num_devices=N_CORES
    )

    xposT = nc.dram_tensor("xposT", [B_LOC, H, T], bf16, kind="ExternalInput").ap()
    xdynT = nc.dram_tensor("xdynT", [B_LOC, H, T], bf16, kind="ExternalInput").ap()
    xposN = nc.dram_tensor("xposN", [B_LOC, 128, T], bf16, kind="ExternalInput").ap()
    # scale_all[p, b, j] = (1 + softplus(beta) * acc_norm)[b, 128*j + p]
    scale_all = nc.dram_tensor("scale_all", [128, B_LOC, NJJ], f32, kind="ExternalInput").ap()
    whT = nc.dram_tensor("whT", [H, H], bf16, kind="ExternalInput").ap()
    wdT = nc.dram_tensor("wdT", [H, H], bf16, kind="ExternalInput").ap()
    wa = nc.dram_tensor("wa", [H, 1], bf16, kind="ExternalInput").ap()
    ident = nc.dram_tensor("ident", [128, 128], f32, kind="ExternalInput").ap()
    ones_col = nc.dram_tensor("ones_col", [128, 1], f32, kind="ExternalInput").ap()
    ones_row = nc.dram_tensor("ones_row", [1, 128], f32, kind="ExternalInput").ap()
    ba_col = nc.dram_tensor("ba_col", [128, 1], f32, kind="ExternalInput").ap()

    alpha_out = nc.dram_tensor("alpha_out", [B_LOC, T], f32, kind="ExternalOutput").ap()
    ctx_out = nc.dram_tensor("ctx_out", [B_LOC, H], f32, kind="ExternalOutput").ap()

    with tile.TileContext(nc) as tc, ExitStack() as ctx:
        consts = ctx.enter_context(tc.tile_pool(name="consts", bufs=1))
        xio_pool = ctx.enter_context(tc.tile_pool(name="xio", bufs=3))
        xn_pool = ctx.enter_context(tc.tile_pool(name="xnp", bufs=1))
        z_pool = ctx.enter_context(tc.tile_pool(name="z", bufs=4))
        sm_pool = ctx.enter_context(tc.tile_pool(name="sm", bufs=4))
        big_pool = ctx.enter_context(tc.tile_pool(name="big", bufs=1))
        ps_pre = ctx.enter_context(tc.tile_pool(name="pspre", bufs=3, space="PSUM"))
        ps_e = ctx.enter_context(tc.tile_pool(name="pse", bufs=2, space="PSUM"))
        ps_sm = ctx.enter_context(tc.tile_pool(name="pssm", bufs=3, space="PSUM"))

        whT_sb = consts.tile([H, H], bf16, tag="whT")
        nc.scalar.dma_start(whT_sb[:], whT[:])
        wdT_sb = consts.tile([H, H], bf16, tag="wdT")
        nc.scalar.dma_start(wdT_sb[:], wdT[:])
        wa_sb = consts.tile([H, 1], bf16, tag="wa")
        nc.scalar.dma_start(wa_sb[:], wa[:])
        ident_sb = consts.tile([128, 128], f32, tag="ident")
        nc.scalar.dma_start(ident_sb[:], ident[:])
        onesc_sb = consts.tile([128, 1], f32, tag="onesc")
        nc.scalar.dma_start(onesc_sb[:], ones_col[:])
        onesr_sb = consts.tile([1, 128], f32, tag="onesr")
        nc.scalar.dma_start(onesr_sb[:], ones_row[:])
        ba_sb = consts.tile([128, 1], f32, tag="ba")
        nc.scalar.dma_start(ba_sb[:], ba_col[:])
        sc_sb = consts.tile([128, B_LOC, NJJ], f32, tag="sc")
        nc.scalar.dma_start(sc_sb[:], scale_all[:])

        # resident X_pos natural-layout for the context matmuls: one big DMA
        xn_sb = xn_pool.tile([128, B_LOC, T], bf16, tag="xn")
        nc.sync.dma_start(
            xn_sb[:], xposN.rearrange("b p u -> p b u")
        )

        es_all = big_pool.tile([128, B_LOC, NJJ], f32, tag="es_all")

        # ---- Phase 1: energies for all samples ----
        n_bat = (n_samples + XBAT - 1) // XBAT
        xt_tiles = {}
        xd_tiles = {}
        for g in range(n_bat):
            lo = g * XBAT
            hi = min(lo + XBAT, n_samples)
            xt = xio_pool.tile([H, XBAT, T], bf16, tag="xt")
            nc.sync.dma_start(
                xt[:, : hi - lo, :], xposT[lo:hi].rearrange("b h t -> h b t")
            )
            xd = xio_pool.tile([H, XBAT, T], bf16, tag="xd")
            nc.sync.dma_start(
                xd[:, : hi - lo, :], xdynT[lo:hi].rearrange("b h t -> h b t")
            )
            xt_tiles[g] = xt
            xd_tiles[g] = xd

        for b in range(n_samples):
            g, r = divmod(b, XBAT)
            xt, xd = xt_tiles[g], xd_tiles[g]
            pe_ps = ps_e.tile([128, NJJ], f32, tag="pe")
            for c in range(NCH):
                pp = ps_pre.tile([128, TCH], f32, tag="pp")
                nc.tensor.matmul(
                    pp[:], lhsT=whT_sb[:], rhs=xt[:, r, c * TCH:(c + 1) * TCH],
                    start=True, stop=False,
                )
                nc.tensor.matmul(
                    pp[:], lhsT=wdT_sb[:], rhs=xd[:, r, c * TCH:(c + 1) * TCH],
                    start=False, stop=True,
                )
                zz = z_pool.tile([128, TCH], bf16, tag="zz")
                nc.scalar.activation(zz[:], pp[:], AF.Tanh)
                for q in range(TCH // 128):
                    jj = c * (TCH // 128) + q
                    nc.tensor.matmul(
                        pe_ps[:, jj:jj + 1],
                        lhsT=zz[:, q * 128:(q + 1) * 128],
                        rhs=wa_sb[:],
                        start=True, stop=True,
                    )
            nc.vector.tensor_copy(es_all[:, b, :], pe_ps[:])

        # ---- Phase 2: batched softmax over [128, n_samples, NJJ] ----
        esb = big_pool.tile([128, B_LOC, NJJ], f32, tag="esb")
        nc.vector.scalar_tensor_tensor(
            out=esb[:, :n_samples, :], in0=es_all[:, :n_samples, :],
            scalar=ba_sb[:], in1=sc_sb[:, :n_samples, :],
            op0=OP.add, op1=OP.mult,
        )
        ex_all = big_pool.tile([128, B_LOC, NJJ], f32, tag="ex_all")
        nc.scalar.activation(ex_all[:, :n_samples, :], esb[:, :n_samples, :], AF.Exp)
        sums = sm_pool.tile([128, B_LOC], f32, tag="sums")
        nc.vector.tensor_reduce(
            out=sums[:, :n_samples], in_=ex_all[:, :n_samples, :],
            axis=mybir.AxisListType.X, op=OP.add,
        )
        ptot = ps_sm.tile([B_LOC, 1], f32, tag="pssm")
        nc.tensor.matmul(
            ptot[: n_samples], lhsT=sums[:, :n_samples], rhs=onesc_sb[:],
            start=True, stop=True,
        )
        tot_sb = sm_pool.tile([B_LOC, 1], f32, tag="tot")
        nc.vector.tensor_copy(tot_sb[: n_samples], ptot[: n_samples])
        # [ns, 1] -> [1, ns]
        ptotr = ps_sm.tile([1, B_LOC], f32, tag="pssm")
        nc.tensor.transpose(
            ptotr[:, :n_samples], tot_sb[: n_samples], ident_sb[: n_samples, : n_samples]
        )
        totr_sb = sm_pool.tile([1, B_LOC], f32, tag="totr")
        nc.vector.tensor_copy(totr_sb[:, :n_samples], ptotr[:, :n_samples])
        # outer product: [128, ns] of per-sample totals on every partition
        pbc = ps_sm.tile([128, B_LOC], f32, tag="pssm")
        nc.tensor.matmul(
            pbc[:, :n_samples], lhsT=onesr_sb[:], rhs=totr_sb[:, :n_samples],
            start=True, stop=True,
        )
        rt = sm_pool.tile([128, B_LOC], f32, tag="rt")
        nc.vector.reciprocal(rt[:, :n_samples], pbc[:, :n_samples])
        al_all = big_pool.tile([128, B_LOC, NJJ], f32, tag="al_all")
        for b in range(n_samples):
            nc.vector.tensor_scalar(
                out=al_all[:, b, :], in0=ex_all[:, b, :],
                scalar1=rt[:, b:b + 1], scalar2=None, op0=OP.mult,
            )
        ab_all = big_pool.tile([128, B_LOC, NJJ], bf16, tag="ab_all")
        nc.vector.tensor_copy(ab_all[:, :n_samples, :], al_all[:, :n_samples, :])

        # ---- Phase 3: alpha.T out + context ----
        for b in range(n_samples):
            pat = ps_sm.tile([NJJ, 128], f32, tag="pssm")
            nc.tensor.transpose(pat[:], al_all[:, b, :], ident_sb[:])
            at = sm_pool.tile([NJJ, 128], f32, tag="at")
            nc.vector.tensor_copy(at[:], pat[:])
            nc.scalar.dma_start(
                alpha_out[b].rearrange("(j u) -> j u", j=NJJ), at[:]
            )
            pctx = ps_sm.tile([1, H], f32, tag="pssm")
            for jj in range(NJJ):
                nc.tensor.matmul(
                    pctx[:],
                    lhsT=ab_all[:, b, jj:jj + 1],
                    rhs=xn_sb[:, b, jj * 128:(jj + 1) * 128],
                    start=(jj == 0), stop=(jj == NJJ - 1),
                )
            ctx_row = sm_pool.tile([1, H], f32, tag="ctxrow")
            nc.vector.tensor_copy(ctx_row[:], pctx[:])
            nc.scalar.dma_start(
                ctx_out[b].rearrange("(o h) -> o h", o=1), ctx_row[:]
            )

    nc.compile()
    return nc


def _get_nc():
    if "nc" not in _cache:
        _cache["nc"] = _build_bass()
    return _cache["nc"]


def _prep_core_inputs(Hp_bf, Hd_bf, scale, b_a):
    """Build the per-core input maps (host-side layout transforms)."""
    ident = np.eye(128, dtype=np.float32)
    ones_col = np.ones((128, 1), np.float32)
    ones_row = np.ones((1, 128), np.float32)
    ba_col = np.full((128, 1), np.float32(b_a), np.float32)
    in_maps = []
    for core in range(N_CORES):
        sl = slice(core * B_LOC, (core + 1) * B_LOC)
        hp = Hp_bf[sl]                       # [16, T, H] bf16
        hd = Hd_bf[sl]
        in_maps.append({
            "xposT": np.ascontiguousarray(hp.transpose(0, 2, 1)),
            "xdynT": np.ascontiguousarray(hd.transpose(0, 2, 1)),
            # [b, (j p), h] -> [b, p, (j h)]
            "xposN": np.ascontiguousarray(
                hp.reshape(B_LOC, NJJ, 128, H).transpose(0, 2, 1, 3)
            ).reshape(B_LOC, 128, T),
            # [b, (j p)] -> [p, b, j]
            "scale_all": np.ascontiguousarray(
                scale[sl].reshape(B_LOC, NJJ, 128).transpose(2, 0, 1)
            ),
            "whT": _cache["whT"],
            "wdT": _cache["wdT"],
            "wa": _cache["wa"],
            "ident": ident,
            "ones_col": ones_col,
            "ones_row": ones_row,
            "ba_col": ba_col,
        })
    return in_maps


def kernel(H_pos, H_dyn, acc_w, W_h, W_d, W_a, b_a, beta):
    from concourse.bass_utils import run_bass_kernel_spmd

    H_pos = np.asarray(H_pos, dtype=np.float32)
    H_dyn = np.asarray(H_dyn, dtype=np.float32)
    acc_w = np.asarray(acc_w, dtype=np.float32)
    W_h = np.asarray(W_h, dtype=np.float32)
    W_d = np.asarray(W_d, dtype=np.float32)
    W_a = np.asarray(W_a, dtype=np.float32)
    b_a_f = float(np.asarray(b_a))
    beta_f = float(np.asarray(beta))

    # host scalar/row prep (tiny): softplus(beta), acc normalization, scale
    beta_pos = float(np.log1p(np.exp(beta_f)))
    acc_norm = acc_w / np.clip(acc_w.max(axis=1, keepdims=True), 1e-6, None)
    scale = (1.0 + beta_pos * acc_norm).astype(np.float32)          # [B, T]

    Hp_bf = H_pos.astype(_BF16)
    Hd_bf = H_dyn.astype(_BF16)
    _cache["whT"] = np.ascontiguousarray(W_h.T).astype(_BF16)
    _cache["wdT"] = np.ascontiguousarray(W_d.T).astype(_BF16)
    _cache["wa"] = W_a.reshape(H, 1).astype(_BF16)

    nc = _get_nc()
    in_maps = _prep_core_inputs(Hp_bf, Hd_bf, scale, b_a_f)
    res = run_bass_kernel_spmd(nc, in_maps, list(range(N_CORES)))
    _cache["last_res"] = res

    alpha = np.concatenate([r["alpha_out"] for r in res.results], axis=0)
    context = np.concatenate([r["ctx_out"] for r in res.results], axis=0)
    return (
        context.astype(np.float32, copy=False),
        alpha.astype(np.float32, copy=False),
    )


# revision 9
# speedup vs baseline: 1.8785x; 1.3346x over previous
"""Additive-attention (tanh energy + softmax + context) kernel for 8 TRN2 NeuronCores.

Data-parallel over the batch dim B=128 -> 16 samples per core.

Group-pipelined per-core schedule (groups of 4 samples, keeps PE dense/warm and
overlaps each group's softmax/context tail with the next group's energy phase):
  energies: pre.T = W_h @ XposT + W_d @ XdynT (PE) -> tanh (ACT) -> energy cols (PE)
  softmax (per group, batched): (e + b_a)*scale (DVE) -> exp (ACT) -> row sums (DVE)
    -> partition totals + broadcast (PE ones-matmuls) -> reciprocal/scale (DVE)
  alpha out: one PE transpose per group [128, 64] -> [64, 128] -> one DMA
  context: ctx.T[h] += Xnat_chunk.T @ alpha_col (PE) -> gathered [128, 16]
  final: one transpose -> [16, 128] -> one DMA

Host side: shard B across 8 cores, cast X to bf16, pre-transpose layouts so every
device DMA is fully contiguous, run SPMD, gather outputs.
"""

import numpy as np
import ml_dtypes

B, T, H = 128, 2048, 128
N_CORES = 8
B_LOC = B // N_CORES      # 16 samples per core
TCH = 512                 # t-chunk for the pre matmuls (one psum bank)
NCH = T // TCH            # 4 chunks per sample
NJJ = T // 128            # 16 energy columns per sample
GRP = 4                   # samples per softmax/ctx group (= DMA batch)

_BF16 = ml_dtypes.bfloat16

_cache = {}


def _build_bass(n_samples=B_LOC):
    import concourse.bass as bass
    import concourse.tile as tile
    from concourse import bacc, mybir
    from contextlib import ExitStack

    f32 = mybir.dt.float32
    bf16 = mybir.dt.bfloat16
    AF = mybir.ActivationFunctionType
    OP = mybir.AluOpType

    assert n_samples % GRP == 0
    n_grp = n_samples // GRP

    nc = bacc.Bacc(
        "TRN2", target_bir_lowering=False, debug=False, num_devices=N_CORES
    )

    xposT = nc.dram_tensor("xposT", [B_LOC, H, T], bf16, kind="ExternalInput").ap()
    xdynT = nc.dram_tensor("xdynT", [B_LOC, H, T], bf16, kind="ExternalInput").ap()
    xposN = nc.dram_tensor("xposN", [B_LOC, 128, T], bf16, kind="ExternalInput").ap()
    # scale_all[p, b, j] = (1 + softplus(beta) * acc_norm)[b, 128*j + p]
    scale_all = nc.dram_tensor("scale_all", [128, B_LOC, NJJ], f32, kind="ExternalInput").ap()
    whT = nc.dram_tensor("whT", [H, H], bf16, kind="ExternalInput").ap()
    wdT = nc.dram_tensor("wdT", [H, H], bf16, kind="ExternalInput").ap()
    wa = nc.dram_tensor("wa", [H, 1], bf16, kind="ExternalInput").ap()
    ident = nc.dram_tensor("ident", [128, 128], f32, kind="ExternalInput").ap()
    ones_col = nc.dram_tensor("ones_col", [128, 1], f32, kind="ExternalInput").ap()
    ones_row = nc.dram_tensor("ones_row", [1, 128], f32, kind="ExternalInput").ap()
    ba_col = nc.dram_tensor("ba_col", [128, 1], f32, kind="ExternalInput").ap()

    alpha_out = nc.dram_tensor("alpha_out", [B_LOC, T], f32, kind="ExternalOutput").ap()
    ctx_out = nc.dram_tensor("ctx_out", [B_LOC, H], f32, kind="ExternalOutput").ap()

    with tile.TileContext(nc) as tc, ExitStack() as ctx:
        consts = ctx.enter_context(tc.tile_pool(name="consts", bufs=1))
        xio_pool = ctx.enter_context(tc.tile_pool(name="xio", bufs=3))
        xn_pool = ctx.enter_context(tc.tile_pool(name="xnp", bufs=1))
        z_pool = ctx.enter_context(tc.tile_pool(name="z", bufs=4))
        sm_pool = ctx.enter_context(tc.tile_pool(name="sm", bufs=4))
        grp_pool = ctx.enter_context(tc.tile_pool(name="grp", bufs=2))
        ps_pre = ctx.enter_context(tc.tile_pool(name="pspre", bufs=4, space="PSUM"))
        ps_e = ctx.enter_context(tc.tile_pool(name="pse", bufs=2, space="PSUM"))
        ps_sm = ctx.enter_context(tc.tile_pool(name="pssm", bufs=2, space="PSUM"))

        # constants on the scalar (ACT) HWDGE ring; bulk X loads on the sync ring
        whT_sb = consts.tile([H, H], bf16, tag="whT")
        nc.scalar.dma_start(whT_sb[:], whT[:])
        wdT_sb = consts.tile([H, H], bf16, tag="wdT")
        nc.scalar.dma_start(wdT_sb[:], wdT[:])
        wa_sb = consts.tile([H, 1], bf16, tag="wa")
        nc.scalar.dma_start(wa_sb[:], wa[:])
        ident_sb = consts.tile([128, 128], f32, tag="ident")
        nc.scalar.dma_start(ident_sb[:], ident[:])
        onesc_sb = consts.tile([128, 1], f32, tag="onesc")
        nc.scalar.dma_start(onesc_sb[:], ones_col[:])
        onesr_sb = consts.tile([1, 128], f32, tag="onesr")
        nc.scalar.dma_start(onesr_sb[:], ones_row[:])
        ba_sb = consts.tile([128, 1], f32, tag="ba")
        nc.scalar.dma_start(ba_sb[:], ba_col[:])
        sc_sb = consts.tile([128, B_LOC, NJJ], f32, tag="sc")
        nc.scalar.dma_start(sc_sb[:], scale_all[:])
        # resident X_pos natural-layout for the context matmuls (needed from
        # the first group's ctx phase onward; scalar ring, overlaps phase 1)
        xn_sb = xn_pool.tile([128, B_LOC, T], bf16, tag="xn")
        nc.scalar.dma_start(xn_sb[:], xposN.rearrange("b p u -> p b u"))

        ctxT_sb = consts.tile([128, B_LOC], f32, tag="ctxT")

        xt_tiles = {}
        xd_tiles = {}
        for g in range(n_grp):
            lo = g * GRP
            xt = xio_pool.tile([H, GRP, T], bf16, tag="xt")
            nc.sync.dma_start(
                xt[:], xposT[lo:lo + GRP].rearrange("b h t -> h b t")
            )
            xd = xio_pool.tile([H, GRP, T], bf16, tag="xd")
            nc.sync.dma_start(
                xd[:], xdynT[lo:lo + GRP].rearrange("b h t -> h b t")
            )
            xt_tiles[g] = xt
            xd_tiles[g] = xd

        for g in range(n_grp):
            xt, xd = xt_tiles[g], xd_tiles[g]
            es_g = grp_pool.tile([128, GRP, NJJ], f32, tag="es_g")
            # ---- energies for the group ----
            for r in range(GRP):
                pe_ps = ps_e.tile([128, NJJ], f32, tag="pe")
                for c in range(NCH):
                    pp = ps_pre.tile([128, TCH], f32, tag="pp")
                    nc.tensor.matmul(
                        pp[:], lhsT=whT_sb[:], rhs=xt[:, r, c * TCH:(c + 1) * TCH],
                        start=True, stop=False,
                    )
                    nc.tensor.matmul(
                        pp[:], lhsT=wdT_sb[:], rhs=xd[:, r, c * TCH:(c + 1) * TCH],
                        start=False, stop=True,
                    )
                    zz = z_pool.tile([128, TCH], bf16, tag="zz")
                    nc.scalar.activation(zz[:], pp[:], AF.Tanh)
                    for q in range(TCH // 128):
                        jj = c * (TCH // 128) + q
                        nc.tensor.matmul(
                            pe_ps[:, jj:jj + 1],
                            lhsT=zz[:, q * 128:(q + 1) * 128],
                            rhs=wa_sb[:],
                            start=True, stop=True,
                        )
                nc.vector.tensor_copy(es_g[:, r, :], pe_ps[:])

            # ---- group softmax ----
            lo = g * GRP
            esb = grp_pool.tile([128, GRP, NJJ], f32, tag="esb")
            nc.vector.scalar_tensor_tensor(
                out=esb[:], in0=es_g[:], scalar=ba_sb[:],
                in1=sc_sb[:, lo:lo + GRP, :], op0=OP.add, op1=OP.mult,
            )
            ex_g = grp_pool.tile([128, GRP, NJJ], f32, tag="ex_g")
            nc.scalar.activation(ex_g[:], esb[:], AF.Exp)
            sums = sm_pool.tile([128, GRP], f32, tag="sums")
            nc.vector.tensor_reduce(
                out=sums[:], in_=ex_g[:], axis=mybir.AxisListType.X, op=OP.add,
            )
            ptot = ps_sm.tile([GRP, 1], f32, tag="pssm")
            nc.tensor.matmul(ptot[:], lhsT=sums[:], rhs=onesc_sb[:], start=True, stop=True)
            tot_sb = sm_pool.tile([GRP, 1], f32, tag="tot")
            nc.vector.tensor_copy(tot_sb[:], ptot[:])
            ptotr = ps_sm.tile([1, GRP], f32, tag="pssm")
            nc.tensor.transpose(ptotr[:], tot_sb[:], ident_sb[:GRP, :GRP])
            totr_sb = sm_pool.tile([1, GRP], f32, tag="totr")
            nc.vector.tensor_copy(totr_sb[:], ptotr[:])
            pbc = ps_sm.tile([128, GRP], f32, tag="pssm")
            nc.tensor.matmul(pbc[:], lhsT=onesr_sb[:], rhs=totr_sb[:], start=True, stop=True)
            rt = sm_pool.tile([128, GRP], f32, tag="rt")
            nc.vector.reciprocal(rt[:], pbc[:])
            al_g = grp_pool.tile([128, GRP, NJJ], f32, tag="al_g")
            for r in range(GRP):
                nc.vector.tensor_scalar(
                    out=al_g[:, r, :], in0=ex_g[:, r, :],
                    scalar1=rt[:, r:r + 1], scalar2=None, op0=OP.mult,
                )
            ab_g = grp_pool.tile([128, GRP, NJJ], bf16, tag="ab_g")
            nc.vector.tensor_copy(ab_g[:], al_g[:])

            # ---- alpha out: one transpose + one DMA per group ----
            pat = ps_sm.tile([GRP * NJJ, 128], f32, tag="pssm")
            nc.tensor.transpose(
                pat[:], al_g[:].rearrange("p r j -> p (r j)"), ident_sb[:]
            )
            at = sm_pool.tile([GRP * NJJ, 128], f32, tag="at")
            nc.vector.tensor_copy(at[:], pat[:])
            nc.scalar.dma_start(
                alpha_out[lo:lo + GRP].rearrange("b (j u) -> (b j) u", j=NJJ), at[:]
            )

            # ---- context: ctx.T columns gathered into ctxT_sb ----
            for r in range(GRP):
                b = lo + r
                pctx = ps_sm.tile([128, 1], f32, tag="pssm")
                for jj in range(NJJ):
                    nc.tensor.matmul(
                        pctx[:],
                        lhsT=xn_sb[:, b, jj * 128:(jj + 1) * 128],
                        rhs=ab_g[:, r, jj:jj + 1],
                        start=(jj == 0), stop=(jj == NJJ - 1),
                    )
                nc.vector.tensor_copy(ctxT_sb[:, b:b + 1], pctx[:])

        # ---- final context transpose + single DMA ----
        pct = ps_sm.tile([B_LOC, 128], f32, tag="pssm")
        nc.tensor.transpose(pct[: n_samples], ctxT_sb[:, :n_samples], ident_sb[:])
        ct = sm_pool.tile([B_LOC, 128], f32, tag="ct")
        nc.vector.tensor_copy(ct[: n_samples], pct[: n_samples])
        nc.scalar.dma_start(ctx_out[: n_samples], ct[: n_samples])

    nc.compile()
    return nc


def _get_nc():
    if "nc" not in _cache:
        _cache["nc"] = _build_bass()
    return _cache["nc"]


def _prep_core_inputs(Hp_bf, Hd_bf, scale, b_a):
    """Build the per-core input maps (host-side layout transforms)."""
    ident = np.eye(128, dtype=np.float32)
    ones_col = np.ones((128, 1), np.float32)
    ones_row = np.ones((1, 128), np.float32)
    ba_col = np.full((128, 1), np.float32(b_a), np.float32)
    in_maps = []
    for core in range(N_CORES):
        sl = slice(core * B_LOC, (core + 1) * B_LOC)
        hp = Hp_bf[sl]                       # [16, T, H] bf16
        hd = Hd_bf[sl]
        in_maps.append({
            "xposT": np.ascontiguousarray(hp.transpose(0, 2, 1)),
            "xdynT": np.ascontiguousarray(hd.transpose(0, 2, 1)),
            # [b, (j p), h] -> [b, p, (j h)]
            "xposN": np.ascontiguousarray(
                hp.reshape(B_LOC, NJJ, 128, H).transpose(0, 2, 1, 3)
            ).reshape(B_LOC, 128, T),
            # [b, (j p)] -> [p, b, j]
            "scale_all": np.ascontiguousarray(
                scale[sl].reshape(B_LOC, NJJ, 128).transpose(2, 0, 1)
            ),
            "whT": _cache["whT"],
            "wdT": _cache["wdT"],
            "wa": _cache["wa"],
            "ident": ident,
            "ones_col": ones_col,
            "ones_row": ones_row,
            "ba_col": ba_col,
        })
    return in_maps


def kernel(H_pos, H_dyn, acc_w, W_h, W_d, W_a, b_a, beta):
    from concourse.bass_utils import run_bass_kernel_spmd

    H_pos = np.asarray(H_pos, dtype=np.float32)
    H_dyn = np.asarray(H_dyn, dtype=np.float32)
    acc_w = np.asarray(acc_w, dtype=np.float32)
    W_h = np.asarray(W_h, dtype=np.float32)
    W_d = np.asarray(W_d, dtype=np.float32)
    W_a = np.asarray(W_a, dtype=np.float32)
    b_a_f = float(np.asarray(b_a))
    beta_f = float(np.asarray(beta))

    # host scalar/row prep (tiny): softplus(beta), acc normalization, scale
    beta_pos = float(np.log1p(np.exp(beta_f)))
    acc_norm = acc_w / np.clip(acc_w.max(axis=1, keepdims=True), 1e-6, None)
    scale = (1.0 + beta_pos * acc_norm).astype(np.float32)          # [B, T]

    Hp_bf = H_pos.astype(_BF16)
    Hd_bf = H_dyn.astype(_BF16)
    _cache["whT"] = np.ascontiguousarray(W_h.T).astype(_BF16)
    _cache["wdT"] = np.ascontiguousarray(W_d.T).astype(_BF16)
    _cache["wa"] = W_a.reshape(H, 1).astype(_BF16)

    nc = _get_nc()
    in_maps = _prep_core_inputs(Hp_bf, Hd_bf, scale, b_a_f)
    res = run_bass_kernel_spmd(nc, in_maps, list(range(N_CORES)))
    _cache["last_res"] = res

    alpha = np.concatenate([r["alpha_out"] for r in res.results], axis=0)
    context = np.concatenate([r["ctx_out"] for r in res.results], axis=0)
    return (
        context.astype(np.float32, copy=False),
        alpha.astype(np.float32, copy=False),
    )


# revision 10
# speedup vs baseline: 2.1899x; 1.1657x over previous
"""Additive-attention (tanh energy + softmax + context) kernel for 8 TRN2 NeuronCores.

Data-parallel over the batch dim B=128 -> 16 samples per core.

Software-pipelined per-core schedule over groups of 4 samples:
    E(g):    pre.T = W_h @ XposT + W_d @ XdynT (PE) -> tanh (ACT) -> energy
             columns (PE) -> es_g (DVE)
    SM(g):   batched group softmax: (e+b_a)*scale (DVE) -> exp (ACT) -> sums
             (DVE) -> totals/broadcast (PE ones-matmuls) -> 1/total, alpha (DVE)
    TAIL(g): alpha.T (one PE transpose + one DMA per group);
             ctx.T[:, b] += Xnat_chunk.T @ alpha_col (PE, psum) -> gather
  emitted as E(0), E(1), SM(0), E(2), SM(1), TAIL(0), ... so the PE stream never
  blocks on the softmax DVE/ACT chain and DMA always has a group in flight.

Host side: shard B across 8 cores, cast X to bf16, pre-transpose layouts so every
device DMA is fully contiguous, run SPMD, gather outputs.
"""

import numpy as np
import ml_dtypes

B, T, H = 128, 2048, 128
N_CORES = 8
B_LOC = B // N_CORES      # 16 samples per core
TCH = 512                 # t-chunk for the pre matmuls (one psum bank)
NCH = T // TCH            # 4 chunks per sample
NJJ = T // 128            # 16 energy columns per sample
GRP = 4                   # samples per softmax/ctx group (= DMA batch)

_BF16 = ml_dtypes.bfloat16

_cache = {}


def _build_bass(n_samples=B_LOC):
    import concourse.bass as bass
    import concourse.tile as tile
    from concourse import bacc, mybir
    from contextlib import ExitStack

    f32 = mybir.dt.float32
    bf16 = mybir.dt.bfloat16
    AF = mybir.ActivationFunctionType
    OP = mybir.AluOpType

    assert n_samples % GRP == 0
    n_grp = n_samples // GRP

    nc = bacc.Bacc(
        "TRN2", target_bir_lowering=False, debug=False, num_devices=N_CORES
    )

    xposT = nc.dram_tensor("xposT", [B_LOC, H, T], bf16, kind="ExternalInput").ap()
    xdynT = nc.dram_tensor("xdynT", [B_LOC, H, T], bf16, kind="ExternalInput").ap()
    xposN = nc.dram_tensor("xposN", [B_LOC, 128, T], bf16, kind="ExternalInput").ap()
    # scale_all[p, b, j] = (1 + softplus(beta) * acc_norm)[b, 128*j + p]
    scale_all = nc.dram_tensor("scale_all", [128, B_LOC, NJJ], f32, kind="ExternalInput").ap()
    whT = nc.dram_tensor("whT", [H, H], bf16, kind="ExternalInput").ap()
    wdT = nc.dram_tensor("wdT", [H, H], bf16, kind="ExternalInput").ap()
    wa = nc.dram_tensor("wa", [H, 1], bf16, kind="ExternalInput").ap()
    ident = nc.dram_tensor("ident", [128, 128], f32, kind="ExternalInput").ap()
    ones_col = nc.dram_tensor("ones_col", [128, 1], f32, kind="ExternalInput").ap()
    ones_row = nc.dram_tensor("ones_row", [1, 128], f32, kind="ExternalInput").ap()
    ba_col = nc.dram_tensor("ba_col", [128, 1], f32, kind="ExternalInput").ap()

    alpha_out = nc.dram_tensor("alpha_out", [B_LOC, T], f32, kind="ExternalOutput").ap()
    ctx_out = nc.dram_tensor("ctx_out", [B_LOC, H], f32, kind="ExternalOutput").ap()

    with tile.TileContext(nc) as tc, ExitStack() as ctx:
        consts = ctx.enter_context(tc.tile_pool(name="consts", bufs=1))
        xio_pool = ctx.enter_context(tc.tile_pool(name="xio", bufs=3))
        xn_pool = ctx.enter_context(tc.tile_pool(name="xnp", bufs=1))
        z_pool = ctx.enter_context(tc.tile_pool(name="z", bufs=4))
        sm_pool = ctx.enter_context(tc.tile_pool(name="sm", bufs=4))
        grp_pool = ctx.enter_context(tc.tile_pool(name="grp", bufs=2))
        ps_pre = ctx.enter_context(tc.tile_pool(name="pspre", bufs=4, space="PSUM"))
        ps_e = ctx.enter_context(tc.tile_pool(name="pse", bufs=2, space="PSUM"))
        ps_sm = ctx.enter_context(tc.tile_pool(name="pssm", bufs=2, space="PSUM"))

        # constants on the scalar (ACT) HWDGE ring; bulk X loads on the sync ring
        whT_sb = consts.tile([H, H], bf16, tag="whT")
        nc.scalar.dma_start(whT_sb[:], whT[:])
        wdT_sb = consts.tile([H, H], bf16, tag="wdT")
        nc.scalar.dma_start(wdT_sb[:], wdT[:])
        wa_sb = consts.tile([H, 1], bf16, tag="wa")
        nc.scalar.dma_start(wa_sb[:], wa[:])
        ident_sb = consts.tile([128, 128], f32, tag="ident")
        nc.scalar.dma_start(ident_sb[:], ident[:])
        onesc_sb = consts.tile([128, 1], f32, tag="onesc")
        nc.scalar.dma_start(onesc_sb[:], ones_col[:])
        onesr_sb = consts.tile([1, 128], f32, tag="onesr")
        nc.scalar.dma_start(onesr_sb[:], ones_row[:])
        ba_sb = consts.tile([128, 1], f32, tag="ba")
        nc.scalar.dma_start(ba_sb[:], ba_col[:])
        sc_sb = consts.tile([128, B_LOC, NJJ], f32, tag="sc")
        nc.scalar.dma_start(sc_sb[:], scale_all[:])

        xn_sb = xn_pool.tile([128, B_LOC, T], bf16, tag="xn")
        ctxT_sb = consts.tile([128, B_LOC], f32, tag="ctxT")

        # ---- input DMAs on the sync ring, phase-1-critical first ----
        xt_tiles = {}
        xd_tiles = {}

        def load_group(g):
            lo = g * GRP
            xt = xio_pool.tile([H, GRP, T], bf16, tag="xt", name=f"xt{g}")
            xd = xio_pool.tile([H, GRP, T], bf16, tag="xd", name=f"xd{g}")
            if g == 0:
                # split first group for a fast pipeline start
                hg = GRP // 2
                nc.sync.dma_start(
                    xt[:, :hg, :], xposT[lo:lo + hg].rearrange("b h t -> h b t"))
                nc.sync.dma_start(
                    xd[:, :hg, :], xdynT[lo:lo + hg].rearrange("b h t -> h b t"))
                nc.sync.dma_start(
                    xt[:, hg:, :],
                    xposT[lo + hg:lo + GRP].rearrange("b h t -> h b t"))
                nc.sync.dma_start(
                    xd[:, hg:, :],
                    xdynT[lo + hg:lo + GRP].rearrange("b h t -> h b t"))
            else:
                nc.sync.dma_start(
                    xt[:], xposT[lo:lo + GRP].rearrange("b h t -> h b t"))
                nc.sync.dma_start(
                    xd[:], xdynT[lo:lo + GRP].rearrange("b h t -> h b t"))
            xt_tiles[g] = xt
            xd_tiles[g] = xd

        def load_xn(g):
            lo = g * GRP
            nc.sync.dma_start(
                xn_sb[:, lo:lo + GRP, :],
                xposN[lo:lo + GRP].rearrange("b p u -> p b u"),
            )

        load_group(0)
        load_group(1)
        load_xn(0)
        load_group(2)
        load_xn(1)
        load_group(3)
        load_xn(2)
        load_xn(3)

        es_tiles = {}
        sm_state = {}

        def energies(g):
            xt, xd = xt_tiles[g], xd_tiles[g]
            es_g = grp_pool.tile([128, GRP, NJJ], f32, tag="es_g", name=f"es{g}")
            for r in range(GRP):
                pe_ps = ps_e.tile([128, NJJ], f32, tag="pe", name=f"pe{g}_{r}")
                for c in range(NCH):
                    pp = ps_pre.tile([128, TCH], f32, tag="pp", name=f"pp{g}_{r}_{c}")
                    nc.tensor.matmul(
                        pp[:], lhsT=whT_sb[:], rhs=xt[:, r, c * TCH:(c + 1) * TCH],
                        start=True, stop=False,
                    )
                    nc.tensor.matmul(
                        pp[:], lhsT=wdT_sb[:], rhs=xd[:, r, c * TCH:(c + 1) * TCH],
                        start=False, stop=True,
                    )
                    zz = z_pool.tile([128, TCH], bf16, tag="zz", name=f"zz{g}_{r}_{c}")
                    nc.scalar.activation(zz[:], pp[:], AF.Tanh)
                    for q in range(TCH // 128):
                        jj = c * (TCH // 128) + q
                        nc.tensor.matmul(
                            pe_ps[:, jj:jj + 1],
                            lhsT=zz[:, q * 128:(q + 1) * 128],
                            rhs=wa_sb[:],
                            start=True, stop=True,
                        )
                nc.vector.tensor_copy(es_g[:, r, :], pe_ps[:])
            es_tiles[g] = es_g

        def softmax(g):
            lo = g * GRP
            es_g = es_tiles[g]
            esb = grp_pool.tile([128, GRP, NJJ], f32, tag="esb", name=f"esb{g}")
            nc.vector.scalar_tensor_tensor(
                out=esb[:], in0=es_g[:], scalar=ba_sb[:],
                in1=sc_sb[:, lo:lo + GRP, :], op0=OP.add, op1=OP.mult,
            )
            ex_g = grp_pool.tile([128, GRP, NJJ], f32, tag="ex_g", name=f"ex{g}")
            nc.scalar.activation(ex_g[:], esb[:], AF.Exp)
            sums = sm_pool.tile([128, GRP], f32, tag="sums", name=f"sums{g}")
            nc.vector.tensor_reduce(
                out=sums[:], in_=ex_g[:], axis=mybir.AxisListType.X, op=OP.add,
            )
            ptot = ps_sm.tile([GRP, 1], f32, tag="pssm", name=f"ptot{g}")
            nc.tensor.matmul(ptot[:], lhsT=sums[:], rhs=onesc_sb[:], start=True, stop=True)
            tot_sb = sm_pool.tile([GRP, 1], f32, tag="tot", name=f"tot{g}")
            nc.vector.tensor_copy(tot_sb[:], ptot[:])
            ptotr = ps_sm.tile([1, GRP], f32, tag="pssm", name=f"ptotr{g}")
            nc.tensor.transpose(ptotr[:], tot_sb[:], ident_sb[:GRP, :GRP])
            totr_sb = sm_pool.tile([1, GRP], f32, tag="totr", name=f"totr{g}")
            nc.vector.tensor_copy(totr_sb[:], ptotr[:])
            pbc = ps_sm.tile([128, GRP], f32, tag="pssm", name=f"pbc{g}")
            nc.tensor.matmul(pbc[:], lhsT=onesr_sb[:], rhs=totr_sb[:], start=True, stop=True)
            rt = sm_pool.tile([128, GRP], f32, tag="rt", name=f"rt{g}")
            nc.vector.reciprocal(rt[:], pbc[:])
            al_g = grp_pool.tile([128, GRP, NJJ], f32, tag="al_g", name=f"al{g}")
            for r in range(GRP):
                nc.vector.tensor_scalar(
                    out=al_g[:, r, :], in0=ex_g[:, r, :],
                    scalar1=rt[:, r:r + 1], scalar2=None, op0=OP.mult,
                )
            ab_g = grp_pool.tile([128, GRP, NJJ], bf16, tag="ab_g", name=f"ab{g}")
            nc.vector.tensor_copy(ab_g[:], al_g[:])
            sm_state[g] = (al_g, ab_g)

        def tail(g):
            lo = g * GRP
            al_g, ab_g = sm_state[g]
            pat = ps_sm.tile([GRP * NJJ, 128], f32, tag="pssm", name=f"pat{g}")
            nc.tensor.transpose(
                pat[:], al_g[:].rearrange("p r j -> p (r j)"), ident_sb[:]
            )
            at = sm_pool.tile([GRP * NJJ, 128], f32, tag="at", name=f"at{g}")
            nc.vector.tensor_copy(at[:], pat[:])
            nc.scalar.dma_start(
                alpha_out[lo:lo + GRP].rearrange("b (j u) -> (b j) u", j=NJJ), at[:]
            )
            for r in range(GRP):
                b = lo + r
                pctx = ps_sm.tile([128, 1], f32, tag="pssm", name=f"pctx{g}_{r}")
                for jj in range(NJJ):
                    nc.tensor.matmul(
                        pctx[:],
                        lhsT=xn_sb[:, b, jj * 128:(jj + 1) * 128],
                        rhs=ab_g[:, r, jj:jj + 1],
                        start=(jj == 0), stop=(jj == NJJ - 1),
                    )
                nc.vector.tensor_copy(ctxT_sb[:, b:b + 1], pctx[:])

        # ---- software pipeline: E(g), SM(g-1), TAIL(g-2) ----
        for g in range(n_grp + 2):
            if g < n_grp:
                energies(g)
            if 0 <= g - 1 < n_grp:
                softmax(g - 1)
            if 0 <= g - 2 < n_grp:
                tail(g - 2)

        # ---- final context transpose + single DMA ----
        pct = ps_sm.tile([B_LOC, 128], f32, tag="pssm", name="pct")
        nc.tensor.transpose(pct[: n_samples], ctxT_sb[:, :n_samples], ident_sb[:])
        ct = sm_pool.tile([B_LOC, 128], f32, tag="ct", name="ct")
        nc.vector.tensor_copy(ct[: n_samples], pct[: n_samples])
        nc.scalar.dma_start(ctx_out[: n_samples], ct[: n_samples])

    nc.compile()
    return nc


def _get_nc():
    if "nc" not in _cache:
        _cache["nc"] = _build_bass()
    return _cache["nc"]


def _prep_core_inputs(Hp_bf, Hd_bf, scale, b_a):
    """Build the per-core input maps (host-side layout transforms)."""
    ident = np.eye(128, dtype=np.float32)
    ones_col = np.ones((128, 1), np.float32)
    ones_row = np.ones((1, 128), np.float32)
    ba_col = np.full((128, 1), np.float32(b_a), np.float32)
    in_maps = []
    for core in range(N_CORES):
        sl = slice(core * B_LOC, (core + 1) * B_LOC)
        hp = Hp_bf[sl]                       # [16, T, H] bf16
        hd = Hd_bf[sl]
        in_maps.append({
            "xposT": np.ascontiguousarray(hp.transpose(0, 2, 1)),
            "xdynT": np.ascontiguousarray(hd.transpose(0, 2, 1)),
            # [b, (j p), h] -> [b, p, (j h)]
            "xposN": np.ascontiguousarray(
                hp.reshape(B_LOC, NJJ, 128, H).transpose(0, 2, 1, 3)
            ).reshape(B_LOC, 128, T),
            # [b, (j p)] -> [p, b, j]
            "scale_all": np.ascontiguousarray(
                scale[sl].reshape(B_LOC, NJJ, 128).transpose(2, 0, 1)
            ),
            "whT": _cache["whT"],
            "wdT": _cache["wdT"],
            "wa": _cache["wa"],
            "ident": ident,
            "ones_col": ones_col,
            "ones_row": ones_row,
            "ba_col": ba_col,
        })
    return in_maps


def kernel(H_pos, H_dyn, acc_w, W_h, W_d, W_a, b_a, beta):
    from concourse.bass_utils import run_bass_kernel_spmd

    H_pos = np.asarray(H_pos, dtype=np.float32)
    H_dyn = np.asarray(H_dyn, dtype=np.float32)
    acc_w = np.asarray(acc_w, dtype=np.float32)
    W_h = np.asarray(W_h, dtype=np.float32)
    W_d = np.asarray(W_d, dtype=np.float32)
    W_a = np.asarray(W_a, dtype=np.float32)
    b_a_f = float(np.asarray(b_a))
    beta_f = float(np.asarray(beta))

    # host scalar/row prep (tiny): softplus(beta), acc normalization, scale
    beta_pos = float(np.log1p(np.exp(beta_f)))
    acc_norm = acc_w / np.clip(acc_w.max(axis=1, keepdims=True), 1e-6, None)
    scale = (1.0 + beta_pos * acc_norm).astype(np.float32)          # [B, T]

    Hp_bf = H_pos.astype(_BF16)
    Hd_bf = H_dyn.astype(_BF16)
    _cache["whT"] = np.ascontiguousarray(W_h.T).astype(_BF16)
    _cache["wdT"] = np.ascontiguousarray(W_d.T).astype(_BF16)
    _cache["wa"] = W_a.reshape(H, 1).astype(_BF16)

    nc = _get_nc()
    in_maps = _prep_core_inputs(Hp_bf, Hd_bf, scale, b_a_f)
    res = run_bass_kernel_spmd(nc, in_maps, list(range(N_CORES)))
    _cache["last_res"] = res

    alpha = np.concatenate([r["alpha_out"] for r in res.results], axis=0)
    context = np.concatenate([r["ctx_out"] for r in res.results], axis=0)
    return (
        context.astype(np.float32, copy=False),
        alpha.astype(np.float32, copy=False),
    )
